# revision 2
# baseline (speedup 1.0000x reference)
"""Trainium2 Bass kernel for nn_MultiHeadAttention_867583393876.

Math (per batch b, head h, all matrices 512x512):
  Qm = x[b] @ WQ[h]; Km = x[b] @ WK[h]
  S  = Qm @ Km            (reference's K.reshape is an identity on a square
                           matrix, so S = Q @ K, not Q @ K^T)
  A  = softmax(S / sqrt(512)) over the QUERY axis t (rows of S)
  out rows of head h: out[b, 64h+u] = sum_j (A @ V)[8u+j] @ WO[512j:512j+512]
                      with V = x[b] @ WV[h]

Key algebraic fold: since A@V@WOj = A@x@(WV@WOj), precompute
  U_j = WV[h] @ WO[512j:512(j+1)]   (8 matrices, once per core)
and per batch compute G = A @ x, then out rows from G and U.  This removes
the V projection entirely: 5 matmul stages per batch instead of 6.

Sharding: head-parallel, core c == head c, no collectives; core c computes
out[:, 64c:64(c+1), :] for all 16 batches.

Precision split:
 - Q/K/ST path in f32r (the scaled logits have sigma ~512, so softmax is an
   argmax; logit noise flips winners, so this path needs the 13-bit mantissa).
 - Value path (G = A@x, U-stage, U precompute) in bf16: A is essentially
   exactly one-hot, so errors are just bf16 quantization of x / WV / WO
   (~0.4% each), far inside the tolerance.  bf16 operands are cast host-side
   (x natural, WV^T, WO), so no on-device rounding copies are needed there.

Everything is computed in transposed space (partition = channel) so the
softmax reduction runs along the free axis.
"""

import numpy as np

B, T, E, H = 16, 512, 512, 8
N_CORES = 8
SCALE = 1.0 / 22.627416997969522  # 1/sqrt(512)

_CACHE = {}


def _emit(ctx, nc, tc, tile, mybir, aps):
    import concourse.bass as bass

    f32 = mybir.dt.float32
    f32r = mybir.dt.float32r
    bf16 = mybir.dt.bfloat16
    x, xb_d, wq, wk, wvt, wo, outp = (
        aps["x"], aps["xb"], aps["wq"], aps["wk"], aps["wvt"], aps["wo"],
        aps["out"],
    )
    ts = bass.ts

    def pool(name, bufs, space="SBUF"):
        return ctx.enter_context(tc.tile_pool(name=name, bufs=bufs, space=space))

    # SBUF pools (sizes are KB/partition; ~208KB usable)
    p_wo = pool("wo", 1)          # 32KB  WO bf16, DMA direct
    p_u = pool("u", 1)            # 32KB  U = WV@WO bf16
    p_w = pool("w", 1)            # 16KB  WQ/WK f32r + 4KB wvT bf16
    p_stage = pool("stage", 2)    # 4KB   fp32 DMA staging for WQ/WK
    p_xn = pool("xn", 2)          # 16KB  x[b]^T fp32 staging
    p_xt = pool("xt", 2)          # 16KB  XT f32r
    p_xb = pool("xb", 2)          # 8KB   x[b] natural bf16 (DMA direct)
    p_q = pool("q", 2)            # 16KB  QmT f32r
    p_k = pool("k", 2)            # 16KB  Km natural f32r
    p_at = pool("at", 2)          # 8KB   AT bf16
    p_scr = pool("scr", 1)        # 2KB   exp scratch fp32
    p_gt = pool("gt", 2)          # 16KB  G^T scrambled, bf16, batch pair
    p_out = pool("ostage", 1)     # 2KB   output staging
    p_small = pool("small", 4)

    ps_mm = pool("ps_mm", 4, space="PSUM")
    ps_st = pool("ps_st", 3, space="PSUM")
    ps_u = pool("ps_u", 1, space="PSUM")

    xload = {}

    # ---- weight loads ----
    def load_round(dram_rows, n_tiles, dst_tile, scope):
        with nc.named_scope(scope):
            for i in range(n_tiles):
                st = p_stage.tile([128, 512], f32, tag="stage")
                nc.sync.dma_start(st[:], dram_rows(i))
                nc.any.tensor_copy(dst_tile[:, ts(i, 512)], st[:])

    # x loads: xT fp32 (rounded to f32r for Q/K) + x natural bf16 (direct)
    def load_x(bb):
        xn = p_xn.tile([128, 4 * 512], f32, tag="xn")
        for i in range(4):
            nc.sync.dma_start(xn[:, ts(i, 512)], x[bb, i * 128:(i + 1) * 128, :])
        xbt = p_xb.tile([128, 4 * 512], bf16, tag="xb")
        for i in range(4):
            nc.sync.dma_start(xbt[:, ts(i, 512)], xb_d[bb, i * 128:(i + 1) * 128, :])
        xload[bb] = (xn, xbt)

    def round_x(bb):
        xn, xbt = xload[bb]
        xt = p_xt.tile([128, 4 * 512], f32r, tag="xt")
        for i in range(4):
            nc.any.tensor_copy(xt[:, ts(i, 512)], xn[:, ts(i, 512)])
        xload[bb] = (xt, xbt)

    wq_r = p_w.tile([128, 4 * 512], f32r, tag="wq")
    load_round(lambda i: wq[i * 128:(i + 1) * 128, :], 4, wq_r, "load_wq")
    load_x(0)
    wk_r = p_w.tile([128, 4 * 512], f32r, tag="wk")
    load_round(lambda i: wk[i * 128:(i + 1) * 128, :], 4, wk_r, "load_wk")

    # wvT + wo: bf16, direct DMA (no rounding copies needed for bf16 matmul)
    wvt_b = p_w.tile([128, 4 * 512], bf16, tag="wvt")
    with nc.named_scope("load_wvt"):
        for i in range(4):
            nc.sync.dma_start(wvt_b[:, ts(i, 512)], wvt[i * 128:(i + 1) * 128, :])
    load_x(1)
    round_x(0)
    wo_b = p_wo.tile([128, 32 * 512], bf16, tag="wo")
    with nc.named_scope("load_wo"):
        for i in range(32):
            nc.sync.dma_start(wo_b[:, ts(i, 512)], wo[i * 128:(i + 1) * 128, :])

    u_r = p_u.tile([128, 32 * 512], bf16, tag="u")

    def col(w, k, blk):
        return w[:, k * 512 + blk * 128: k * 512 + blk * 128 + 128]

    def mm_acc(ps_tile, pairs):
        for i, (l, r) in enumerate(pairs):
            nc.tensor.matmul(
                ps_tile[:], l, r,
                start=(i == 0), stop=(i == len(pairs) - 1),
            )

    # ---- U_j = WV @ WO_j precompute (bf16), emitted in chunks from batches ----
    def emit_u_pre(j):
        with nc.named_scope(f"upre{j}"):
            for wblk in range(4):
                pu = ps_mm.tile([128, 512], f32, tag="mm")
                mm_acc(pu, [(col(wvt_b, k, wblk), wo_b[:, ts(j * 4 + k, 512)])
                            for k in range(4)])
                nc.any.tensor_copy(u_r[:, ts(j * 4 + wblk, 512)], pu[:])

    gt_state = [None]
    pending = []

    def emit_g(b, xbt, at):
        # ---- G^T = x-natural^T-contract @ AT (bf16) ----
        # PSUM->SBUF copy scatters into the U-ready scrambled layout:
        # free index = vblk*1024 + j*128 + half*64 + u  where the G column
        # t = 8u + j and half = b%2.
        if b % 2 == 0:
            gtp = p_gt.tile([128, 2 * 4 * 512], bf16, tag="gt")
            gt_state[0] = gtp
        gt = gt_state[0]
        half = b % 2
        for vblk in range(4):
            pg = ps_mm.tile([128, 512], f32, tag="mm")
            mm_acc(pg, [(col(xbt, m, vblk), at[:, ts(m, 512)]) for m in range(4)])
            gf = gt[:]
            pf = pg[:]
            dst = bass.AP(
                gf.tensor, gf.offset + vblk * 1024 + half * 64,
                [list(gf.ap[0]), [1, 64], [128, 8]],
            )
            src = bass.AP(
                pf.tensor, pf.offset, [list(pf.ap[0]), [8, 64], [1, 8]]
            )
            nc.vector.tensor_copy(dst, src)

        # ---- U stage for the (b-1, b) pair ----
        if b % 2 == 1:
            po = ps_u.tile([128, 512], f32, tag="ups")
            for kt in range(32):
                j, dblk = kt // 4, kt % 4
                rhs = u_r[:, ts(kt, 512)]
                lhs = gt[:, dblk * 1024 + j * 128: dblk * 1024 + (j + 1) * 128]
                nc.tensor.matmul(
                    po[:], lhs, rhs,
                    start=(kt == 0), stop=(kt == 31),
                )
            so = p_out.tile([128, 512], f32, tag="so")
            nc.any.tensor_copy(so[:], po[:])
            nc.sync.dma_start(outp[b - 1], so[0:64, :])
            nc.sync.dma_start(outp[b], so[64:128, :])

    for b in range(B):
        with nc.named_scope(f"batch{b}"):
            if b + 1 < B:
                load_x(b + 1)
            xt, xbt = xload.pop(b)

            # ---- QmT = WQ^T @ XT (f32r), scale folded into rounding copy ----
            qt = p_q.tile([128, 4 * 512], f32r, tag="q")
            for dblk in range(4):
                pq = ps_mm.tile([128, 512], f32, tag="mm")
                mm_acc(pq, [(col(wq_r, k, dblk), xt[:, ts(k, 512)]) for k in range(4)])
                nc.any.tensor_scalar_mul(qt[:, ts(dblk, 512)], pq[:], SCALE)

            # ---- Km natural = XT^T-contract @ WK (f32r) ----
            km = p_k.tile([128, 4 * 512], f32r, tag="k")
            for tblk in range(4):
                pk = ps_mm.tile([128, 512], f32, tag="mm")
                mm_acc(pk, [(col(xt, k, tblk), wk_r[:, ts(k, 512)]) for k in range(4)])
                nc.any.tensor_copy(km[:, ts(tblk, 512)], pk[:])

            # round next batch's xT while this batch's PE work runs
            if b + 1 < B:
                round_x(b + 1)

            # deferred G + U-stage of the previous batch: its softmax inputs
            # are a batch old, so the PE never stalls on them
            if pending:
                emit_g(*pending.pop())

            # U precompute interleaved into the first two batches
            if b in (0, 1):
                for j in range(4 * b, 4 * b + 4):
                    emit_u_pre(j)

            # ---- ST = Km^T-contract @ QmT, softmax along free axis ----
            at = p_at.tile([128, 4 * 512], bf16, tag="at")
            for sblk in range(4):
                pst = ps_st.tile([128, 512], f32, tag="st")
                mm_acc(pst, [(col(km, m, sblk), qt[:, ts(m, 512)]) for m in range(4)])
                nmx = p_small.tile([128, 1], f32, tag="nmx")
                nc.vector.tensor_reduce(
                    nmx[:], pst[:], axis=mybir.AxisListType.X,
                    op=mybir.AluOpType.max, negate=True,
                )
                scr = p_scr.tile([128, 512], f32, tag="scr")
                sm = p_small.tile([128, 1], f32, tag="sm")
                nc.scalar.activation(
                    scr[:], pst[:], mybir.ActivationFunctionType.Exp,
                    bias=nmx[:], scale=1.0, accum_out=sm[:],
                )
                rc = p_small.tile([128, 1], f32, tag="rc")
                nc.vector.reciprocal(rc[:], sm[:])
                nc.vector.tensor_scalar_mul(at[:, ts(sblk, 512)], scr[:], rc[:])

            pending.append((b, xbt, at))

    emit_g(*pending.pop())


def _build():
    import concourse.bass as bass  # noqa: F401
    import concourse.tile as tile
    from concourse import bacc, mybir

    nc = bacc.Bacc(
        "TRN2",
        target_bir_lowering=False,
        debug=False,
        enable_asserts=False,
        num_devices=N_CORES,
    )
    f32 = mybir.dt.float32
    bf16 = mybir.dt.bfloat16
    aps = {
        "x": nc.dram_tensor("x", (B, E, T), f32, kind="ExternalInput").ap(),
        "xb": nc.dram_tensor("xb", (B, T, E), bf16, kind="ExternalInput").ap(),
        "wq": nc.dram_tensor("wq", (E, E), f32, kind="ExternalInput").ap(),
        "wk": nc.dram_tensor("wk", (E, E), f32, kind="ExternalInput").ap(),
        "wvt": nc.dram_tensor("wvt", (E, E), bf16, kind="ExternalInput").ap(),
        "wo": nc.dram_tensor("wo", (H * E, E), bf16, kind="ExternalInput").ap(),
        "out": nc.dram_tensor("out", (B, 64, E), f32, kind="ExternalOutput").ap(),
    }
    from contextlib import ExitStack

    with tile.TileContext(nc) as tc, ExitStack() as ctx:
        _emit(ctx, nc, tc, tile, mybir, aps)
    nc.compile()
    return nc


def _get_nc():
    if "nc" not in _CACHE:
        _CACHE["nc"] = _build()
    return _CACHE["nc"]


def run(inputs, trace=False):
    import ml_dtypes
    from concourse.bass_utils import run_bass_kernel_spmd

    nc = _get_nc()
    bf16 = ml_dtypes.bfloat16
    x = np.asarray(inputs["x"], dtype=np.float32)
    xT = np.ascontiguousarray(x.transpose(0, 2, 1))
    xB = np.ascontiguousarray(x.astype(bf16))
    WQ = np.asarray(inputs["WQ"], dtype=np.float32)
    WK = np.asarray(inputs["WK"], dtype=np.float32)
    WV = np.asarray(inputs["WV"], dtype=np.float32)
    WO = np.asarray(inputs["WO"], dtype=np.float32).astype(bf16)
    WO = np.ascontiguousarray(WO)
    in_maps = [
        {
            "x": xT,
            "xb": xB,
            "wq": np.ascontiguousarray(WQ[c]),
            "wk": np.ascontiguousarray(WK[c]),
            "wvt": np.ascontiguousarray(WV[c].T.astype(bf16)),
            "wo": WO,
        }
        for c in range(N_CORES)
    ]
    res = run_bass_kernel_spmd(
        nc, in_maps, core_ids=list(range(N_CORES)), trace=trace
    )
    out = np.empty((B, T, E), dtype=np.float32)
    for c in range(N_CORES):
        out[:, 64 * c:64 * (c + 1), :] = res.results[c]["out"]
    return out, res


def kernel(**inputs):
    out, _ = run(inputs, trace=False)
    return out
